# revision 22
# baseline (speedup 1.0000x reference)
"""Trainium2 Bass kernel for nn_CrossAttention (B=1, T=4, N=2048, D=256, H=8, hd=32).

Sharding: 8 cores = (t in 0..3) x (query half in 0..1). Each core computes full
attention for its 1024 queries over all 2048 keys of its t, all 8 heads, then the
output projection for its query rows. No collectives.

Layout strategy (per core, "S^T" transposed-scores streaming):
  - PE-transpose x, ctx, W once -> x^T, ctx^T, W^T on SBUF.
  - q^T = Wq^T.T @ x^T   [256, 1024] (head-dim on partitions, grouped 4 heads/128)
  - k^T = Wk^T.T @ ctx^T [256, 2048]
  - v    = ctx @ Wv^T     [2048, 256] (natural)
  - per (head-group g, query-block qb, key-tile kt):
      S^T tile [128 keys, 512 q] via row-tiled (K=32) 4-head-concurrent matmuls
      P^T = exp(SCALE * S^T): split between ACT (exact exp) and DVE (custom
          polynomial-exp op, one instruction)
      AV: out^T += v_h.T @ P^T via col-tiled (M=32) 4-head-concurrent matmuls
      sums: r_h += ones.T @ P^T via 16-tile (32x32) matmul batches
  - epilogue: 1/r via fast-reciprocal, broadcast via K=1 diagonal-tiled matmuls,
      oc^T = out^T * (1/r)
  - y = oc^T.T @ Wo^T + bo  [1024, 256] (natural rows) -> DMA out.

Matmul operands are bitcast to float32r (1 cycle/row at free-dim >= 256).
"""

import numpy as np

import concourse.bass as bass
import concourse.mybir as mybir
import concourse.tile as tile
from concourse.vector_clock import ScopedClock
from concourse.masks import make_identity
from concourse.alu_op_type import AluOpType

# ----------------------------------------------------------------------------
# constants
# ----------------------------------------------------------------------------
B, T, N, D = 1, 4, 2048, 256
H, HD = 8, 32
SCALE = HD ** -0.5
NQ = N // 2          # queries per core
N_CORES = 8
QB = 512             # query block (matmul free dim)
KT = 128             # key tile (contraction tile)
N_QB = NQ // QB      # 2
N_KT = N // KT       # 16
N_G = 2              # head groups of 4

FP32 = mybir.dt.float32
FP32R = mybir.dt.float32r
BF16 = mybir.dt.bfloat16

# polynomial exp: exp(u) ~= (c0 + c1 u + c2 u^2 + c3 u^3)^2 for u in [-1.35, 1.35]
# (fit of exp(u/2); relative error <= 2.2e-3). Input is the RAW score s with
# u = SCALE * s folded into the coefficients.
_PC = (0.9990915219933622, 0.5011579162173042, 0.1295924724493572,
       0.020361888770054334)
PB0 = float(_PC[0])
PB1 = float(_PC[1] * SCALE)
PB2 = float(_PC[2] * SCALE ** 2)
PB3 = float(_PC[3] * SCALE ** 3)

# DVE exp offload: mode and how many key-tiles of head 3 go to DVE
# modes: "poly4" (4 DVE ops, accurate), "schraudolph" (1 DVE op, ~1% err), "off"
import os as _os
DVE_EXP_MODE = _os.environ.get("DVE_EXP_MODE", "poly4")
DVE_KT_CUT = int(_os.environ.get("DVE_KT_CUT", "13"))

# factorization of the cubic b0+b1 s+b2 s^2+b3 s^3 (s = raw score) as
# c3*(s+R1)*((s+BT)(s+R1)+GM); the c3^2 factor cancels in the softmax.
def _factor_cubic():
    r = np.roots([PB3, PB2, PB1, PB0])
    real = [z for z in r if abs(z.imag) < 1e-9][0].real
    pair = [z for z in r if abs(z.imag) >= 1e-9][0]
    r1 = -real
    beta, gamq = -2 * pair.real, abs(pair) ** 2
    bt = beta - r1
    return float(r1), float(bt), float(gamq - bt * r1)

R1, BT, GM = _factor_cubic()
# fold sqrt(c3) into the linear factor so the chain computes the cubic
# exactly (matching the ACT heads' scale):
#   u = SC*(s+R1); w = (s+BT)*u; m = (w + SC*GM)*u = cubic(s); P = m^2
SC = float(np.sqrt(PB3))
GM2 = float(SC * GM)

# Schraudolph: int32(SCH_A * s + SCH_B) bitcast to fp32 ~ exp(SCALE*s)
SCH_A16 = float((1 << 7) / np.log(2) * SCALE)
SCH_B16 = float((127 << 7) - 0.0437 * (1 << 7))

# engine used for PSUM->SBUF copies of big intermediates
# 'act' | 'dve'
COPY_ENGINES = {
    "xT": "dve", "cT": "dve", "WT": "dve",
    "qT": "act", "kT": "act", "v": "dve", "y": "act",
}


# ----------------------------------------------------------------------------
# TileContext with the final-drain sem-wait overflow split (walrus in this
# toolchain accepts only a few waits on a sequencer instruction).
# ----------------------------------------------------------------------------
MAX_DRAIN_WAITS = 1


class TC(tile.TileContext):
    def _split_all_waits(self):
        """Walrus in this toolchain rejects instructions with more than one
        sync wait. Move overflow waits onto same-engine nop carriers placed
        immediately before the instruction in program order."""
        nc = self.nc
        cur_insts = nc.cur_bb.bb.instructions
        for f in nc.m.functions:
            for bb in f.blocks:
                insts = bb.instructions
                i = 0
                while i < len(insts):
                    inst = insts[i]
                    si = inst.sync_info
                    if si is None or not si.on_wait or \
                            len(si.on_wait) <= MAX_DRAIN_WAITS:
                        i += 1
                        continue
                    waits = list(si.on_wait)
                    si.on_wait = waits[:MAX_DRAIN_WAITS]
                    extra = waits[MAX_DRAIN_WAITS:]
                    carriers = []
                    for j in range(0, len(extra), MAX_DRAIN_WAITS):
                        nop = nc.engines[inst.engine].nop(nofuse=True)
                        nop.ins.sync_info = mybir.SyncInfo(
                            on_wait=extra[j:j + MAX_DRAIN_WAITS], on_update=[]
                        )
                        # nop() appended to cur_bb; relocate it
                        popped = cur_insts.pop()
                        assert popped.name == nop.ins.name
                        carriers.append(nop.ins)
                    for k, c in enumerate(carriers):
                        insts.insert(i + k, c)
                    i += len(carriers) + 1

    def _drain_and_barrier(self, tick_clock, wait_clock):
        nc = self.nc
        self._split_all_waits()
        drain_inst = nc.sync.drain()
        wait_clock.add_sem_waits(
            drain_inst.ins, ScopedClock({None: tick_clock.global_clock})
        )
        si = drain_inst.ins.sync_info
        if si is not None and si.on_wait and len(si.on_wait) > MAX_DRAIN_WAITS:
            waits = list(si.on_wait)
            si.on_wait = waits[:MAX_DRAIN_WAITS]
            extra = waits[MAX_DRAIN_WAITS:]
            bb = nc.cur_bb.bb
            assert bb.instructions[-1].name == drain_inst.ins.name
            bb.instructions.pop()
            for i in range(0, len(extra), MAX_DRAIN_WAITS):
                nop = nc.sync.nop(nofuse=True)
                nop.ins.sync_info = mybir.SyncInfo(
                    on_wait=extra[i:i + MAX_DRAIN_WAITS], on_update=[]
                )
            bb.add_instruction(drain_inst.ins)

        nc.all_engine_barrier()
        assert self.sems is not None
        popped = nc._tile_sem_poison_stack.pop()
        assert popped is self._sem_poison
        nc.clear_and_free_semaphores(list(self.sems.allocated().values()))
        nc.all_engine_barrier()


# ----------------------------------------------------------------------------
# custom DVE op: out = (C0 + x*(C1 + x*(C2 + x*Src1)))^2  (Src1 = [P,1] = b3)
# ----------------------------------------------------------------------------
_POLY_OP = None


def _register_poly_exp_op():
    global _POLY_OP
    if _POLY_OP is not None:
        return _POLY_OP
    import concourse.dve_ops as dve_ops
    from concourse.dve_spec import C0, C1, C2, Spec, Src0, Src1, _has_src1, lower, sq
    from concourse.dve_uop import DveOpSpec

    name = "POLY_EXP_SQ_ANT"

    def _ref(in0, in1, s0, s1, imm2):
        x = in0.astype(np.float32)
        b3 = in1.astype(np.float32)  # [P, 1]
        p = (s0 + x * (s1 + x * (imm2 + x * b3))).astype(np.float32)
        return (p * p).astype(np.float32)

    spec = Spec(
        body=sq(C0 + Src0 * (C1 + Src0 * (C2 + Src0 * Src1))),
        reference=_ref,
    )
    if name not in dve_ops._SUB_OPCODE_FOR_NAME:
        row = max(dve_ops._SUB_OPCODE_FOR_NAME.values()) + 1
        assert row < 0x20
        op = dve_ops.DveOp(name, spec, subdim=False, uops_sha={})
        dve_ops.OPS.append(op)
        dve_ops._SUB_OPCODE_FOR_NAME[name] = row
        dve_ops.CUSTOM_DVE_SPECS[name] = spec
        for ver in ("v3", "v4"):
            uops = lower(spec, ver=ver)
            sha = DveOpSpec(
                name=name, opcode=row, uops=uops, rd1_en=_has_src1(spec)
            ).sha(ver)
            op.uops_sha[ver] = sha
        _POLY_OP = op
    else:
        for op in dve_ops.OPS:
            if op.name == name:
                _POLY_OP = op
    return _POLY_OP


def _r(ap):
    """bitcast an AP to float32r for matmul operands"""
    return ap.bitcast(FP32R)


def _copy(nc, engine, out, in_):
    if engine == "act":
        nc.scalar.copy(out, in_)
    else:
        nc.vector.tensor_copy(out=out, in_=in_)


# ----------------------------------------------------------------------------
# kernel program (one core)
# ----------------------------------------------------------------------------
def build_nc():
    nc = bass.Bass()

    xq_d = nc.dram_tensor("xq", [NQ, D], FP32, kind="ExternalInput")
    ctx_d = nc.dram_tensor("ctx", [N, D], FP32, kind="ExternalInput")
    w_d = {}
    for w in ("Wq", "Wk", "Wv", "Wo"):
        w_d[w] = nc.dram_tensor(w, [D, D], FP32, kind="ExternalInput")
    b_d = {}
    for b in ("bq", "bk", "bv", "bo"):
        b_d[b] = nc.dram_tensor(b, [1, D], FP32, kind="ExternalInput")
    cst_d = nc.dram_tensor("cst_ones", [1, QB], FP32, kind="ExternalInput")
    y_d = nc.dram_tensor("y", [NQ, D], FP32, kind="ExternalOutput")

    with TC(nc) as tc:
        _build_body(nc, tc, xq_d, ctx_d, w_d, b_d, y_d, cst_d)
    return nc


def _build_body(nc, tc, xq_d, ctx_d, w_d, b_d, y_d, cst_d):
    from contextlib import ExitStack
    ctx_stack = ExitStack()
    persist = ctx_stack.enter_context(tc.tile_pool(name="persist", bufs=1))
    loads = ctx_stack.enter_context(tc.tile_pool(name="loads", bufs=2))
    # single PSUM pool, 8 banks: s0..s3, av, sums, rbc (+1 spare)
    psum = ctx_stack.enter_context(tc.tile_pool(name="psum", bufs=1, space="PSUM"))
    pexp = ctx_stack.enter_context(tc.tile_pool(name="pexp", bufs=2))
    pexpt = ctx_stack.enter_context(tc.tile_pool(name="pexpt", bufs=2))
    ytmp = ctx_stack.enter_context(tc.tile_pool(name="ytmp", bufs=2))

    _tp_ctr = [0]

    def ptile_rot(tags):
        """rotating psum tile over the given tag list"""
        t = psum.tile([128, QB], FP32, tag=tags[_tp_ctr[0] % len(tags)],
                      name=f"ps_{_tp_ctr[0]}")
        _tp_ctr[0] += 1
        return t

    # --- persistent SBUF tensors ---
    ident = persist.tile([128, 128], FP32)
    make_identity(nc, ident)
    ones_all = persist.tile([1, QB], FP32R)     # all-ones row (from DRAM)
    nc.sync.dma_start(out=ones_all, in_=_r(cst_d[:, :]))
    ones_blkf = persist.tile([128, 32], FP32)
    nc.vector.memset(ones_blkf, 1.0)
    ones_blk = persist.tile([128, 32], BF16)    # all-ones block for sums
    nc.vector.tensor_copy(out=ones_blk, in_=ones_blkf)

    xT = [persist.tile([128, NQ], FP32R, tag=f"xT{k}", name=f"xT{k}") for k in range(2)]
    cT = [persist.tile([128, N], FP32R, tag=f"cT{k}", name=f"cT{k}") for k in range(2)]
    WT = {w: [persist.tile([128, D], FP32R, tag=f"{w}T{k}", name=f"{w}T{k}") for k in range(2)]
          for w in ("Wq", "Wk", "Wv", "Wo")}
    qT = [persist.tile([128, NQ], FP32R, tag=f"qT{g}", name=f"qT{g}") for g in range(N_G)]
    kTt = [persist.tile([128, N], FP32R, tag=f"kT{g}", name=f"kT{g}") for g in range(N_G)]
    v_sb = [persist.tile([128, D], BF16, tag=f"v{kt}", name=f"v{kt}") for kt in range(N_KT)]
    ocT = [persist.tile([128, NQ], FP32R, tag=f"ocT{g}", name=f"ocT{g}") for g in range(N_G)]
    bias_sb = {}
    for bname in ("bq", "bk", "bv", "bo"):
        t = persist.tile([1, D], FP32R, tag=bname, name=f"b_{bname}")
        nc.sync.dma_start(out=t, in_=_r(b_d[bname][:, :]))
        bias_sb[bname] = t
    scr = persist.tile([128, QB], FP32)   # ln(r)
    scr2 = persist.tile([128, QB], FP32)  # 1/r

    # --- load + transpose x, ctx, W ---
    def load_transpose(dram, nrows, dest, name):
        # dram [nrows, 256] -> dest[kd] [128, nrows] for kd in 0,1
        eng = COPY_ENGINES[name]
        for it in range(nrows // 128):
            t_in = loads.tile([128, D], FP32, tag=f"ld_{name}")
            nc.sync.dma_start(out=t_in, in_=dram[it * 128:(it + 1) * 128, :])
            for kd in range(2):
                pt = ptile_rot(("s0", "s1", "s2", "s3"))
                nc.tensor.transpose(
                    pt[:, 0:128], t_in[:, kd * 128:(kd + 1) * 128], ident)
                _copy(nc, eng, dest[kd][:, it * 128:(it + 1) * 128],
                      pt[:, 0:128])

    load_transpose(xq_d, NQ, xT, "xT")
    load_transpose(ctx_d, N, cT, "cT")
    for w in ("Wq", "Wk", "Wv", "Wo"):
        load_transpose(w_d[w], D, WT[w], "WT")

    # --- projections ---
    # q^T [g][128, NQ] = Wq^T.T @ x^T + bq
    for g in range(N_G):
        for qb in range(N_QB):
            pp = ptile_rot(("av", "sums"))
            nc.tensor.matmul(pp, _r(bias_sb["bq"][0:1, g * 128:(g + 1) * 128]),
                             ones_all[0:1, :], start=True, stop=False)
            for kd in range(2):
                nc.tensor.matmul(pp, _r(WT["Wq"][kd][:, g * 128:(g + 1) * 128]),
                                 _r(xT[kd][:, qb * QB:(qb + 1) * QB]),
                                 start=False, stop=(kd == 1))
            _copy(nc, COPY_ENGINES["qT"], qT[g][:, qb * QB:(qb + 1) * QB], pp)

    # k^T [g][128, N] = Wk^T.T @ ctx^T + bk
    for g in range(N_G):
        for kb in range(N // QB):
            pp = ptile_rot(("av", "sums"))
            nc.tensor.matmul(pp, _r(bias_sb["bk"][0:1, g * 128:(g + 1) * 128]),
                             ones_all[0:1, :], start=True, stop=False)
            for kd in range(2):
                nc.tensor.matmul(pp, _r(WT["Wk"][kd][:, g * 128:(g + 1) * 128]),
                                 _r(cT[kd][:, kb * QB:(kb + 1) * QB]),
                                 start=False, stop=(kd == 1))
            _copy(nc, COPY_ENGINES["kT"], kTt[g][:, kb * QB:(kb + 1) * QB], pp)

    # v [kt][128, 256] = ctx @ Wv^T + bv   (natural layout)
    for kt in range(N_KT):
        pp = ptile_rot(("av", "sums"))
        nc.tensor.matmul(pp[:, 0:D], ones_all[0:1, 0:128],
                         _r(bias_sb["bv"][0:1, :]), start=True, stop=False)
        for kd in range(2):
            nc.tensor.matmul(pp[:, 0:D], _r(cT[kd][:, kt * 128:(kt + 1) * 128]),
                             _r(WT["Wv"][kd][:, :]), start=False, stop=(kd == 1))
        _copy(nc, COPY_ENGINES["v"], v_sb[kt], pp[:, 0:D])

    # --- attention ---
    for g in range(N_G):
        for qb in range(N_QB):
            po = psum.tile([128, QB], FP32, tag="av")       # AV accumulator
            pr = psum.tile([128, QB], FP32, tag="sums")     # rowsum accumulator
            for kt in range(N_KT):
                ps = [psum.tile([128, QB], FP32, tag=f"s{h}",
                                name=f"s_{g}_{qb}_{kt}_{h}") for h in range(4)]
                ptile = [pexp.tile([128, QB], BF16, tag=f"pT{h}",
                                   name=f"pT_{g}_{qb}_{kt}_{h}") for h in range(4)]
                for h in range(4):
                    # S^T = k_h.T.T @ q_h^T  (K=32, row-tiled)
                    nc.tensor.matmul(
                        ps[h],
                        _r(kTt[g][32 * h:32 * h + 32, kt * KT:(kt + 1) * KT]),
                        _r(qT[g][32 * h:32 * h + 32, qb * QB:(qb + 1) * QB]),
                        start=True, stop=True, tile_position=(32 * h, 0),
                    )
                for h in range(4):
                    on_dve = (DVE_EXP_MODE != "off" and h == 3
                              and kt < DVE_KT_CUT)
                    if on_dve and DVE_EXP_MODE == "schraudolph":
                        nc.vector.tensor_scalar(
                            out=ptile[h].bitcast(mybir.dt.int16),
                            in0=ps[h], scalar1=SCH_A16, scalar2=SCH_B16,
                            op0=AluOpType.mult, op1=AluOpType.add)
                    elif on_dve:  # poly4: P = ((s+R1)((s+BT)(s+R1)+GM))^2
                        ut = pexpt.tile([128, QB], FP32, tag="u",
                                        name=f"u_{g}_{qb}_{kt}")
                        wt = pexpt.tile([128, QB], FP32, tag="w",
                                        name=f"w_{g}_{qb}_{kt}")
                        mt = pexpt.tile([128, QB], FP32, tag="m",
                                        name=f"m_{g}_{qb}_{kt}")
                        nc.vector.tensor_scalar(
                            out=ut, in0=ps[h], scalar1=R1, scalar2=SC,
                            op0=AluOpType.add, op1=AluOpType.mult)
                        nc.vector.scalar_tensor_tensor(
                            out=wt, in0=ps[h], scalar=BT, in1=ut,
                            op0=AluOpType.add, op1=AluOpType.mult)
                        nc.vector.scalar_tensor_tensor(
                            out=mt, in0=wt, scalar=GM2, in1=ut,
                            op0=AluOpType.add, op1=AluOpType.mult)
                        nc.vector.tensor_mul(ptile[h], mt, mt)
                    else:
                        nc.scalar.activation(
                            ptile[h], ps[h],
                            mybir.ActivationFunctionType.Exp, scale=SCALE,
                        )
                for h in range(4):
                    # AV: col-tiled M=32. PSUM group start/stop only on the
                    # first/last matmul touching this bank; first write to an
                    # element overwrites (has_written), later ones accumulate.
                    nc.tensor.matmul(
                        po[32 * h:32 * h + 32, :],
                        v_sb[kt][:, g * 128 + 32 * h: g * 128 + 32 * h + 32],
                        ptile[h][:, :],
                        start=(kt == 0), stop=(kt == N_KT - 1),
                        tile_position=(0, 32 * h), skip_group_check=True,
                    )
                for h in range(4):
                    # rowsums, col-tiled like AV: all-ones K=128 lhsT
                    # replicates each head's key-sum across its 32 rows
                    # (the 1/r broadcast for free).
                    nc.tensor.matmul(
                        pr[32 * h:32 * h + 32, :],
                        ones_blk[:, :],
                        ptile[h][:, :],
                        start=(kt == 0), stop=(kt == N_KT - 1),
                        tile_position=(0, 32 * h), skip_group_check=True,
                    )
            # epilogue: oc^T[:, qb] = po * (1/r); pr already holds r
            # replicated across each head's 32 rows. 1/r = exp(-ln r) on ACT
            # (custom DVE ops don't compile in this toolchain).
            nc.scalar.activation(scr, pr, mybir.ActivationFunctionType.Ln)
            nc.scalar.activation(scr2, scr,
                                 mybir.ActivationFunctionType.Exp, scale=-1.0)
            nc.vector.tensor_mul(ocT[g][:, qb * QB:(qb + 1) * QB], po, scr2)

    # --- output projection: y = oc^T.T @ Wo^T + bo ---
    for qt in range(NQ // 128):
        pp = ptile_rot(("av", "sums"))
        nc.tensor.matmul(pp[:, 0:D], ones_all[0:1, 0:128],
                         _r(bias_sb["bo"][0:1, :]), start=True, stop=False)
        for kd in range(2):
            nc.tensor.matmul(pp[:, 0:D], _r(ocT[kd][:, qt * 128:(qt + 1) * 128]),
                             _r(WT["Wo"][kd][:, :]), start=False, stop=(kd == 1))
        yt = ytmp.tile([128, D], FP32, tag="y")
        _copy(nc, COPY_ENGINES["y"], yt, pp[:, 0:D])
        nc.sync.dma_start(out=y_d[qt * 128:(qt + 1) * 128, :], in_=yt)

    ctx_stack.close()


# ----------------------------------------------------------------------------
# host entry point
# ----------------------------------------------------------------------------
_NC_CACHE = None


def _get_nc():
    global _NC_CACHE
    if _NC_CACHE is None:
        _NC_CACHE = build_nc()
    return _NC_CACHE


def make_in_maps(x, context, Wq, bq, Wk, bk, Wv, bv, Wo, bo):
    in_maps = []
    for c in range(N_CORES):
        t, half = c // 2, c % 2
        in_maps.append({
            "xq": np.ascontiguousarray(x[0, t, half * NQ:(half + 1) * NQ]),
            "ctx": np.ascontiguousarray(context[0, t]),
            "Wq": np.asarray(Wq), "Wk": np.asarray(Wk),
            "Wv": np.asarray(Wv), "Wo": np.asarray(Wo),
            "bq": np.asarray(bq).reshape(1, D),
            "bk": np.asarray(bk).reshape(1, D),
            "bv": np.asarray(bv).reshape(1, D),
            "bo": np.asarray(bo).reshape(1, D),
            "cst_ones": np.ones((1, QB), dtype=np.float32),
        })
    return in_maps


def kernel(x, context, Wq, bq, Wk, bk, Wv, bv, Wo, bo):
    from concourse.bass_utils import run_bass_kernel_spmd

    nc = _get_nc()
    in_maps = make_in_maps(x, context, Wq, bq, Wk, bk, Wv, bv, Wo, bo)
    res = run_bass_kernel_spmd(nc, in_maps, list(range(N_CORES)))
    out = np.zeros((B, T, N, D), dtype=np.float32)
    for c in range(N_CORES):
        t, half = c // 2, c % 2
        out[0, t, half * NQ:(half + 1) * NQ] = res.results[c]["y"]
    return out


# revision 26
# speedup vs baseline: 4375.7026x; 4375.7026x over previous
"""Trainium2 Bass kernel for nn_CrossAttention (B=1, T=4, N=2048, D=256, H=8, hd=32).

Sharding: 8 cores = (t in 0..3) x (query half in 0..1). Each core computes full
attention for its 1024 queries over all 2048 keys of its t, all 8 heads, then the
output projection for its query rows. No collectives.

Layout strategy (per core, "S^T" transposed-scores streaming):
  - PE-transpose x, ctx, W once -> x^T, ctx^T, W^T on SBUF.
  - q^T = Wq^T.T @ x^T   [256, 1024] (head-dim on partitions, grouped 4 heads/128)
  - k^T = Wk^T.T @ ctx^T [256, 2048]
  - v    = ctx @ Wv^T     [2048, 256] (natural)
  - per (head-group g, query-block qb, key-tile kt):
      S^T tile [128 keys, 512 q] via row-tiled (K=32) 4-head-concurrent matmuls
      P^T = exp(SCALE * S^T): split between ACT (exact exp) and DVE (custom
          polynomial-exp op, one instruction)
      AV: out^T += v_h.T @ P^T via col-tiled (M=32) 4-head-concurrent matmuls
      sums: r_h += ones.T @ P^T via 16-tile (32x32) matmul batches
  - epilogue: 1/r via fast-reciprocal, broadcast via K=1 diagonal-tiled matmuls,
      oc^T = out^T * (1/r)
  - y = oc^T.T @ Wo^T + bo  [1024, 256] (natural rows) -> DMA out.

Matmul operands are bitcast to float32r (1 cycle/row at free-dim >= 256).
"""

import numpy as np

import concourse.bass as bass
import concourse.mybir as mybir
import concourse.tile as tile
from concourse.vector_clock import ScopedClock
from concourse.masks import make_identity
from concourse.alu_op_type import AluOpType

# ----------------------------------------------------------------------------
# constants
# ----------------------------------------------------------------------------
B, T, N, D = 1, 4, 2048, 256
H, HD = 8, 32
SCALE = HD ** -0.5
NQ = N // 2          # queries per core
N_CORES = 8
QB = 512             # query block (matmul free dim)
KT = 128             # key tile (contraction tile)
N_QB = NQ // QB      # 2
N_KT = N // KT       # 16
N_G = 2              # head groups of 4

FP32 = mybir.dt.float32
FP32R = mybir.dt.float32r
BF16 = mybir.dt.bfloat16

# polynomial exp: exp(u) ~= (c0 + c1 u + c2 u^2 + c3 u^3)^2 for u in [-1.35, 1.35]
# (fit of exp(u/2); relative error <= 2.2e-3). Input is the RAW score s with
# u = SCALE * s folded into the coefficients.
_PC = (0.9990915219933622, 0.5011579162173042, 0.1295924724493572,
       0.020361888770054334)
PB0 = float(_PC[0])
PB1 = float(_PC[1] * SCALE)
PB2 = float(_PC[2] * SCALE ** 2)
PB3 = float(_PC[3] * SCALE ** 3)

# DVE exp offload: mode and how many key-tiles of head 3 go to DVE
# modes: "poly4" (4 DVE ops, accurate), "schraudolph" (1 DVE op, ~1% err), "off"
import os as _os
DVE_EXP_MODE = _os.environ.get("DVE_EXP_MODE", "off")
DVE_KT_CUT = int(_os.environ.get("DVE_KT_CUT", "13"))
# repeat the whole body in a HW loop (timing experiments only)
BODY_REPEAT = int(_os.environ.get("BODY_REPEAT", "1"))

# factorization of the cubic b0+b1 s+b2 s^2+b3 s^3 (s = raw score) as
# c3*(s+R1)*((s+BT)(s+R1)+GM); the c3^2 factor cancels in the softmax.
def _factor_cubic():
    r = np.roots([PB3, PB2, PB1, PB0])
    real = [z for z in r if abs(z.imag) < 1e-9][0].real
    pair = [z for z in r if abs(z.imag) >= 1e-9][0]
    r1 = -real
    beta, gamq = -2 * pair.real, abs(pair) ** 2
    bt = beta - r1
    return float(r1), float(bt), float(gamq - bt * r1)

R1, BT, GM = _factor_cubic()
# fold sqrt(c3) into the linear factor so the chain computes the cubic
# exactly (matching the ACT heads' scale):
#   u = SC*(s+R1); w = (s+BT)*u; m = (w + SC*GM)*u = cubic(s); P = m^2
SC = float(np.sqrt(PB3))
GM2 = float(SC * GM)

# Schraudolph: int32(SCH_A * s + SCH_B) bitcast to fp32 ~ exp(SCALE*s)
SCH_A16 = float((1 << 7) / np.log(2) * SCALE)
SCH_B16 = float((127 << 7) - 0.0437 * (1 << 7))

# engine used for PSUM->SBUF copies of big intermediates
# 'act' | 'dve'
COPY_ENGINES = {
    "xT": "dve", "cT": "dve", "WT": "dve",
    "qT": "act", "kT": "act", "v": "dve", "y": "act",
}


# ----------------------------------------------------------------------------
# TileContext with the final-drain sem-wait overflow split (walrus in this
# toolchain accepts only a few waits on a sequencer instruction).
# ----------------------------------------------------------------------------
MAX_DRAIN_WAITS = 1


class TC(tile.TileContext):
    def _split_all_waits(self):
        """Walrus in this toolchain rejects instructions with more than one
        sync wait. Move overflow waits onto same-engine nop carriers placed
        immediately before the instruction in program order."""
        nc = self.nc
        cur_insts = nc.cur_bb.bb.instructions
        for f in nc.m.functions:
            for bb in f.blocks:
                insts = bb.instructions
                i = 0
                while i < len(insts):
                    inst = insts[i]
                    si = inst.sync_info
                    if si is None or not si.on_wait or \
                            len(si.on_wait) <= MAX_DRAIN_WAITS:
                        i += 1
                        continue
                    waits = list(si.on_wait)
                    si.on_wait = waits[:MAX_DRAIN_WAITS]
                    extra = waits[MAX_DRAIN_WAITS:]
                    carriers = []
                    for j in range(0, len(extra), MAX_DRAIN_WAITS):
                        nop = nc.engines[inst.engine].nop(nofuse=True)
                        nop.ins.sync_info = mybir.SyncInfo(
                            on_wait=extra[j:j + MAX_DRAIN_WAITS], on_update=[]
                        )
                        # nop() appended to cur_bb; relocate it
                        popped = cur_insts.pop()
                        assert popped.name == nop.ins.name
                        carriers.append(nop.ins)
                    for k, c in enumerate(carriers):
                        insts.insert(i + k, c)
                    i += len(carriers) + 1

    def _drain_and_barrier(self, tick_clock, wait_clock):
        nc = self.nc
        self._split_all_waits()
        drain_inst = nc.sync.drain()
        wait_clock.add_sem_waits(
            drain_inst.ins, ScopedClock({None: tick_clock.global_clock})
        )
        si = drain_inst.ins.sync_info
        if si is not None and si.on_wait and len(si.on_wait) > MAX_DRAIN_WAITS:
            waits = list(si.on_wait)
            si.on_wait = waits[:MAX_DRAIN_WAITS]
            extra = waits[MAX_DRAIN_WAITS:]
            bb = nc.cur_bb.bb
            assert bb.instructions[-1].name == drain_inst.ins.name
            bb.instructions.pop()
            for i in range(0, len(extra), MAX_DRAIN_WAITS):
                nop = nc.sync.nop(nofuse=True)
                nop.ins.sync_info = mybir.SyncInfo(
                    on_wait=extra[i:i + MAX_DRAIN_WAITS], on_update=[]
                )
            bb.add_instruction(drain_inst.ins)

        nc.all_engine_barrier()
        assert self.sems is not None
        popped = nc._tile_sem_poison_stack.pop()
        assert popped is self._sem_poison
        nc.clear_and_free_semaphores(list(self.sems.allocated().values()))
        nc.all_engine_barrier()


# ----------------------------------------------------------------------------
# custom DVE op: out = (C0 + x*(C1 + x*(C2 + x*Src1)))^2  (Src1 = [P,1] = b3)
# ----------------------------------------------------------------------------
_POLY_OP = None


def _register_poly_exp_op():
    global _POLY_OP
    if _POLY_OP is not None:
        return _POLY_OP
    import concourse.dve_ops as dve_ops
    from concourse.dve_spec import C0, C1, C2, Spec, Src0, Src1, _has_src1, lower, sq
    from concourse.dve_uop import DveOpSpec

    name = "POLY_EXP_SQ_ANT"

    def _ref(in0, in1, s0, s1, imm2):
        x = in0.astype(np.float32)
        b3 = in1.astype(np.float32)  # [P, 1]
        p = (s0 + x * (s1 + x * (imm2 + x * b3))).astype(np.float32)
        return (p * p).astype(np.float32)

    spec = Spec(
        body=sq(C0 + Src0 * (C1 + Src0 * (C2 + Src0 * Src1))),
        reference=_ref,
    )
    if name not in dve_ops._SUB_OPCODE_FOR_NAME:
        row = max(dve_ops._SUB_OPCODE_FOR_NAME.values()) + 1
        assert row < 0x20
        op = dve_ops.DveOp(name, spec, subdim=False, uops_sha={})
        dve_ops.OPS.append(op)
        dve_ops._SUB_OPCODE_FOR_NAME[name] = row
        dve_ops.CUSTOM_DVE_SPECS[name] = spec
        for ver in ("v3", "v4"):
            uops = lower(spec, ver=ver)
            sha = DveOpSpec(
                name=name, opcode=row, uops=uops, rd1_en=_has_src1(spec)
            ).sha(ver)
            op.uops_sha[ver] = sha
        _POLY_OP = op
    else:
        for op in dve_ops.OPS:
            if op.name == name:
                _POLY_OP = op
    return _POLY_OP


def _r(ap):
    """bitcast an AP to float32r for matmul operands"""
    return ap.bitcast(FP32R)


def _copy(nc, engine, out, in_):
    if engine == "act":
        nc.scalar.copy(out, in_)
    else:
        nc.vector.tensor_copy(out=out, in_=in_)


# ----------------------------------------------------------------------------
# kernel program (one core)
# ----------------------------------------------------------------------------
def build_nc():
    nc = bass.Bass()

    xq_d = nc.dram_tensor("xq", [NQ, D], FP32, kind="ExternalInput")
    ctx_d = nc.dram_tensor("ctx", [N, D], FP32, kind="ExternalInput")
    w_d = {}
    for w in ("Wq", "Wk", "Wv", "Wo"):
        w_d[w] = nc.dram_tensor(w, [D, D], FP32, kind="ExternalInput")
    b_d = {}
    for b in ("bq", "bk", "bv", "bo"):
        b_d[b] = nc.dram_tensor(b, [1, D], FP32, kind="ExternalInput")
    cst_d = nc.dram_tensor("cst_ones", [1, QB], FP32, kind="ExternalInput")
    y_d = nc.dram_tensor("y", [NQ, D], FP32, kind="ExternalOutput")

    with TC(nc) as tc:
        if BODY_REPEAT > 1:
            with tc.For_i(0, BODY_REPEAT, 1):
                _build_body(nc, tc, xq_d, ctx_d, w_d, b_d, y_d, cst_d)
        else:
            _build_body(nc, tc, xq_d, ctx_d, w_d, b_d, y_d, cst_d)
    return nc


def _build_body(nc, tc, xq_d, ctx_d, w_d, b_d, y_d, cst_d):
    from contextlib import ExitStack
    ctx_stack = ExitStack()
    persist = ctx_stack.enter_context(tc.tile_pool(name="persist", bufs=1))
    loads = ctx_stack.enter_context(tc.tile_pool(name="loads", bufs=2))
    # single PSUM pool, 8 banks: s0..s3, av, sums, rbc (+1 spare)
    psum = ctx_stack.enter_context(tc.tile_pool(name="psum", bufs=1, space="PSUM"))
    pexp = ctx_stack.enter_context(tc.tile_pool(name="pexp", bufs=2))
    pexpt = ctx_stack.enter_context(tc.tile_pool(name="pexpt", bufs=2))
    ytmp = ctx_stack.enter_context(tc.tile_pool(name="ytmp", bufs=2))

    _tp_ctr = [0]

    def ptile_rot(tags):
        """rotating psum tile over the given tag list"""
        t = psum.tile([128, QB], FP32, tag=tags[_tp_ctr[0] % len(tags)],
                      name=f"ps_{_tp_ctr[0]}")
        _tp_ctr[0] += 1
        return t

    # --- persistent SBUF tensors ---
    ident = persist.tile([128, 128], FP32)
    make_identity(nc, ident)
    ones_all = persist.tile([1, QB], FP32R)     # all-ones row (from DRAM)
    nc.sync.dma_start(out=ones_all, in_=_r(cst_d[:, :]))
    ones_blkf = persist.tile([128, 32], FP32)
    nc.vector.memset(ones_blkf, 1.0)
    ones_blk = persist.tile([128, 32], BF16)    # all-ones block for sums
    nc.vector.tensor_copy(out=ones_blk, in_=ones_blkf)

    xT = [persist.tile([128, NQ], FP32R, tag=f"xT{k}", name=f"xT{k}") for k in range(2)]
    cT = [persist.tile([128, N], FP32R, tag=f"cT{k}", name=f"cT{k}") for k in range(2)]
    WT = {w: [persist.tile([128, D], FP32R, tag=f"{w}T{k}", name=f"{w}T{k}") for k in range(2)]
          for w in ("Wq", "Wk", "Wv", "Wo")}
    qT = [persist.tile([128, NQ], FP32R, tag=f"qT{g}", name=f"qT{g}") for g in range(N_G)]
    kTt = [persist.tile([128, N], FP32R, tag=f"kT{g}", name=f"kT{g}") for g in range(N_G)]
    v_sb = [persist.tile([128, D], BF16, tag=f"v{kt}", name=f"v{kt}") for kt in range(N_KT)]
    ocT = [persist.tile([128, NQ], FP32R, tag=f"ocT{g}", name=f"ocT{g}") for g in range(N_G)]
    bias_sb = {}
    for bname in ("bq", "bk", "bv", "bo"):
        t = persist.tile([1, D], FP32R, tag=bname, name=f"b_{bname}")
        nc.sync.dma_start(out=t, in_=_r(b_d[bname][:, :]))
        bias_sb[bname] = t
    scr = persist.tile([128, QB], FP32)   # ln(r)
    scr2 = persist.tile([128, QB], FP32)  # 1/r

    # --- load + transpose x, ctx, W ---
    def load_transpose(dram, nrows, dest, name):
        # dram [nrows, 256] -> dest[kd] [128, nrows] for kd in 0,1
        eng = COPY_ENGINES[name]
        for it in range(nrows // 128):
            t_in = loads.tile([128, D], FP32, tag=f"ld_{name}")
            nc.sync.dma_start(out=t_in, in_=dram[it * 128:(it + 1) * 128, :])
            for kd in range(2):
                pt = ptile_rot(("s0", "s1", "s2", "s3"))
                nc.tensor.transpose(
                    pt[:, 0:128], t_in[:, kd * 128:(kd + 1) * 128], ident)
                _copy(nc, eng, dest[kd][:, it * 128:(it + 1) * 128],
                      pt[:, 0:128])

    load_transpose(xq_d, NQ, xT, "xT")
    load_transpose(ctx_d, N, cT, "cT")
    for w in ("Wq", "Wk", "Wv", "Wo"):
        load_transpose(w_d[w], D, WT[w], "WT")

    # --- projections ---
    # q^T [g][128, NQ] = Wq^T.T @ x^T + bq
    for g in range(N_G):
        for qb in range(N_QB):
            pp = ptile_rot(("av", "sums"))
            nc.tensor.matmul(pp, _r(bias_sb["bq"][0:1, g * 128:(g + 1) * 128]),
                             ones_all[0:1, :], start=True, stop=False)
            for kd in range(2):
                nc.tensor.matmul(pp, _r(WT["Wq"][kd][:, g * 128:(g + 1) * 128]),
                                 _r(xT[kd][:, qb * QB:(qb + 1) * QB]),
                                 start=False, stop=(kd == 1))
            _copy(nc, COPY_ENGINES["qT"], qT[g][:, qb * QB:(qb + 1) * QB], pp)

    # k^T [g][128, N] = Wk^T.T @ ctx^T + bk
    for g in range(N_G):
        for kb in range(N // QB):
            pp = ptile_rot(("av", "sums"))
            nc.tensor.matmul(pp, _r(bias_sb["bk"][0:1, g * 128:(g + 1) * 128]),
                             ones_all[0:1, :], start=True, stop=False)
            for kd in range(2):
                nc.tensor.matmul(pp, _r(WT["Wk"][kd][:, g * 128:(g + 1) * 128]),
                                 _r(cT[kd][:, kb * QB:(kb + 1) * QB]),
                                 start=False, stop=(kd == 1))
            _copy(nc, COPY_ENGINES["kT"], kTt[g][:, kb * QB:(kb + 1) * QB], pp)

    # v [kt][128, 256] = ctx @ Wv^T + bv   (natural layout)
    for kt in range(N_KT):
        pp = ptile_rot(("av", "sums"))
        nc.tensor.matmul(pp[:, 0:D], ones_all[0:1, 0:128],
                         _r(bias_sb["bv"][0:1, :]), start=True, stop=False)
        for kd in range(2):
            nc.tensor.matmul(pp[:, 0:D], _r(cT[kd][:, kt * 128:(kt + 1) * 128]),
                             _r(WT["Wv"][kd][:, :]), start=False, stop=(kd == 1))
        _copy(nc, COPY_ENGINES["v"], v_sb[kt], pp[:, 0:D])

    # --- attention ---
    for g in range(N_G):
        for qb in range(N_QB):
            po = psum.tile([128, QB], FP32, tag="av")       # AV accumulator
            pr = psum.tile([128, QB], FP32, tag="sums")     # rowsum accumulator
            for kt in range(N_KT):
                ps = [psum.tile([128, QB], FP32, tag=f"s{h}",
                                name=f"s_{g}_{qb}_{kt}_{h}") for h in range(4)]
                ptile = [pexp.tile([128, QB], BF16, tag=f"pT{h}",
                                   name=f"pT_{g}_{qb}_{kt}_{h}") for h in range(4)]
                for h in range(4):
                    # S^T = k_h.T.T @ q_h^T  (K=32, row-tiled)
                    nc.tensor.matmul(
                        ps[h],
                        _r(kTt[g][32 * h:32 * h + 32, kt * KT:(kt + 1) * KT]),
                        _r(qT[g][32 * h:32 * h + 32, qb * QB:(qb + 1) * QB]),
                        start=True, stop=True, tile_position=(32 * h, 0),
                    )
                for h in range(4):
                    on_dve = (DVE_EXP_MODE != "off" and h == 3
                              and kt < DVE_KT_CUT)
                    if on_dve and DVE_EXP_MODE == "schraudolph":
                        nc.vector.tensor_scalar(
                            out=ptile[h].bitcast(mybir.dt.int16),
                            in0=ps[h], scalar1=SCH_A16, scalar2=SCH_B16,
                            op0=AluOpType.mult, op1=AluOpType.add)
                    elif on_dve:  # poly4: P = ((s+R1)((s+BT)(s+R1)+GM))^2
                        ut = pexpt.tile([128, QB], FP32, tag="u",
                                        name=f"u_{g}_{qb}_{kt}")
                        wt = pexpt.tile([128, QB], FP32, tag="w",
                                        name=f"w_{g}_{qb}_{kt}")
                        mt = pexpt.tile([128, QB], FP32, tag="m",
                                        name=f"m_{g}_{qb}_{kt}")
                        nc.vector.tensor_scalar(
                            out=ut, in0=ps[h], scalar1=R1, scalar2=SC,
                            op0=AluOpType.add, op1=AluOpType.mult)
                        nc.vector.scalar_tensor_tensor(
                            out=wt, in0=ps[h], scalar=BT, in1=ut,
                            op0=AluOpType.add, op1=AluOpType.mult)
                        nc.vector.scalar_tensor_tensor(
                            out=mt, in0=wt, scalar=GM2, in1=ut,
                            op0=AluOpType.add, op1=AluOpType.mult)
                        nc.vector.tensor_mul(ptile[h], mt, mt)
                    else:
                        nc.scalar.activation(
                            ptile[h], ps[h],
                            mybir.ActivationFunctionType.Exp, scale=SCALE,
                        )
                for h in range(4):
                    # AV: col-tiled M=32. PSUM group start/stop only on the
                    # first/last matmul touching this bank; first write to an
                    # element overwrites (has_written), later ones accumulate.
                    nc.tensor.matmul(
                        po[32 * h:32 * h + 32, :],
                        v_sb[kt][:, g * 128 + 32 * h: g * 128 + 32 * h + 32],
                        ptile[h][:, :],
                        start=(kt == 0), stop=(kt == N_KT - 1),
                        tile_position=(0, 32 * h), skip_group_check=True,
                    )
                for h in range(4):
                    # rowsums, col-tiled like AV: all-ones K=128 lhsT
                    # replicates each head's key-sum across its 32 rows
                    # (the 1/r broadcast for free).
                    nc.tensor.matmul(
                        pr[32 * h:32 * h + 32, :],
                        ones_blk[:, :],
                        ptile[h][:, :],
                        start=(kt == 0), stop=(kt == N_KT - 1),
                        tile_position=(0, 32 * h), skip_group_check=True,
                    )
            # epilogue: oc^T[:, qb] = po * (1/r); pr already holds r
            # replicated across each head's 32 rows. 1/r = exp(-ln r) on ACT
            # (custom DVE ops don't compile in this toolchain).
            nc.scalar.activation(scr, pr, mybir.ActivationFunctionType.Ln)
            nc.scalar.activation(scr2, scr,
                                 mybir.ActivationFunctionType.Exp, scale=-1.0)
            nc.vector.tensor_mul(ocT[g][:, qb * QB:(qb + 1) * QB], po, scr2)

    # --- output projection: y = oc^T.T @ Wo^T + bo ---
    for qt in range(NQ // 128):
        pp = ptile_rot(("av", "sums"))
        nc.tensor.matmul(pp[:, 0:D], ones_all[0:1, 0:128],
                         _r(bias_sb["bo"][0:1, :]), start=True, stop=False)
        for kd in range(2):
            nc.tensor.matmul(pp[:, 0:D], _r(ocT[kd][:, qt * 128:(qt + 1) * 128]),
                             _r(WT["Wo"][kd][:, :]), start=False, stop=(kd == 1))
        yt = ytmp.tile([128, D], FP32, tag="y")
        _copy(nc, COPY_ENGINES["y"], yt, pp[:, 0:D])
        nc.sync.dma_start(out=y_d[qt * 128:(qt + 1) * 128, :], in_=yt)

    ctx_stack.close()


# ----------------------------------------------------------------------------
# host entry point
# ----------------------------------------------------------------------------
_NC_CACHE = None


def _get_nc():
    global _NC_CACHE
    if _NC_CACHE is None:
        _NC_CACHE = build_nc()
    return _NC_CACHE


def make_in_maps(x, context, Wq, bq, Wk, bk, Wv, bv, Wo, bo):
    in_maps = []
    for c in range(N_CORES):
        t, half = c // 2, c % 2
        in_maps.append({
            "xq": np.ascontiguousarray(x[0, t, half * NQ:(half + 1) * NQ]),
            "ctx": np.ascontiguousarray(context[0, t]),
            "Wq": np.asarray(Wq), "Wk": np.asarray(Wk),
            "Wv": np.asarray(Wv), "Wo": np.asarray(Wo),
            "bq": np.asarray(bq).reshape(1, D),
            "bk": np.asarray(bk).reshape(1, D),
            "bv": np.asarray(bv).reshape(1, D),
            "bo": np.asarray(bo).reshape(1, D),
            "cst_ones": np.ones((1, QB), dtype=np.float32),
        })
    return in_maps


def kernel(x, context, Wq, bq, Wk, bk, Wv, bv, Wo, bo):
    from concourse.bass_utils import run_bass_kernel_spmd

    nc = _get_nc()
    in_maps = make_in_maps(x, context, Wq, bq, Wk, bk, Wv, bv, Wo, bo)
    res = run_bass_kernel_spmd(nc, in_maps, list(range(N_CORES)))
    out = np.zeros((B, T, N, D), dtype=np.float32)
    for c in range(N_CORES):
        t, half = c // 2, c % 2
        out[0, t, half * NQ:(half + 1) * NQ] = res.results[c]["y"]
    return out
